# revision 9
# baseline (speedup 1.0000x reference)
"""Trainium2 Bass kernel for nn_DictionaryWiseModel.

Reference computation (per notebook b):
    mask[c,l]  = src[b,c] <= l <= end[b,c]
    pooled     = (mask @ feature[b]) / counts          # [C, H]
    logits     = pooled @ fc_weight.T + fc_bias        # [C, 1]
Output: logits stacked over b -> [B*C, 1].

Strategy: data-parallel over B across 8 cores (1 notebook per core).
Per core:
  - span masks built on-chip: int32 iota of l values vs DMA-broadcast
    src/end columns, mask = (l >= src) - (l > end), written as float32r
  - the big einsum runs on the tensor engine as 16 accumulating matmuls
    (mask chunk [128,64] stationary, feature chunk [128,512]x2 moving)
    in float32r (1 cycle/row vs 4 for fp32; mask is exact 0/1 so only
    the feature mantissa rounds)
  - fc contraction: one fused scalar_tensor_tensor with accum_out,
    then *1/count + bias, DMA out [64, 1]
The kernel is HBM-bound: 8 MB of feature per core (~23 us at 358 GB/s).
"""

import numpy as np

B, L, H, C = 8, 2048, 1024, 64
NCH = L // 128  # 16 l-chunks of 128

_CACHE = {}


def _build_nc():
    import concourse.bacc as bacc
    import concourse.mybir as mybir
    import concourse.tile as tile

    f32 = mybir.dt.float32
    f32r = mybir.dt.float32r
    i32 = mybir.dt.int32
    Alu = mybir.AluOpType

    nc = bacc.Bacc("TRN2", target_bir_lowering=False, debug=False)

    # float32r: same bits as f32 host-side; PE matmuls run 4x faster.
    feat = nc.dram_tensor("feature", [L, H], f32r, kind="ExternalInput")
    pos = nc.dram_tensor("pos", [C, 2], i32, kind="ExternalInput")
    fcw = nc.dram_tensor("fc_w", [1, H], f32, kind="ExternalInput")
    fcb = nc.dram_tensor("fc_b", [1, 1], f32, kind="ExternalInput")
    outd = nc.dram_tensor("out", [C, 1], f32, kind="ExternalOutput")

    with tile.TileContext(nc) as tc:
        with (
            tc.tile_pool(name="setup", bufs=1) as setup,
            tc.tile_pool(name="featp", bufs=8) as featp,
            tc.tile_pool(name="acc", bufs=1, space="PSUM") as accp,
        ):
            # ---- span masks: mask[p, i, c] = (l >= src[c]) - (l > end[c]),
            # with l = 128*i + p ----
            src_b = setup.tile([128, C], i32)
            nc.sync.dma_start(
                src_b[:], pos[:, 0:1].rearrange("c o -> o c").broadcast_to((128, C))
            )
            end_b = setup.tile([128, C], i32)
            nc.sync.dma_start(
                end_b[:], pos[:, 1:2].rearrange("c o -> o c").broadcast_to((128, C))
            )
            iota_t = setup.tile([128, NCH * C], i32)
            iota_r = iota_t[:].rearrange("p (i c) -> p i c", i=NCH)
            nc.gpsimd.iota(
                iota_r, pattern=[[128, NCH], [0, C]], base=0, channel_multiplier=1
            )
            ge_t = setup.tile([128, NCH * C], f32)
            ge_r = ge_t[:].rearrange("p (i c) -> p i c", i=NCH)
            src_bb = src_b[:].rearrange("p (o c) -> p o c", o=1).broadcast_to((128, NCH, C))
            nc.vector.tensor_tensor(ge_r, iota_r, src_bb, Alu.is_ge)
            gt_t = setup.tile([128, NCH * C], f32)
            gt_r = gt_t[:].rearrange("p (i c) -> p i c", i=NCH)
            end_bb = end_b[:].rearrange("p (o c) -> p o c", o=1).broadcast_to((128, NCH, C))
            nc.vector.tensor_tensor(gt_r, iota_r, end_bb, Alu.is_gt)
            mask_t = setup.tile([128, NCH * C], f32r)
            mask_r = mask_t[:].rearrange("p (i c) -> p i c", i=NCH)
            nc.vector.tensor_tensor(mask_r, ge_r, gt_r, Alu.subtract)

            # ---- main loop: pooled[c, h] += mask_i^T @ F_i ----
            pooled = accp.tile([C, H], f32)
            for i in range(NCH):
                ft = featp.tile([128, H], f32r)
                nc.sync.dma_start(ft[:], feat[i * 128 : (i + 1) * 128, :])
                for half in range(2):
                    nc.tensor.matmul(
                        pooled[:, half * 512 : (half + 1) * 512],
                        mask_r[:, i, :],
                        ft[:, half * 512 : (half + 1) * 512],
                        start=(i == 0),
                        stop=(i == NCH - 1),
                    )

            # ---- epilogue inputs (all off the critical path) ----
            pos_sb = setup.tile([C, 2], i32)
            nc.sync.dma_start(pos_sb[:], pos[:])
            w64 = setup.tile([C, H], f32)
            nc.sync.dma_start(w64[:], fcw[:1, :].broadcast_to((C, H)))
            b64 = setup.tile([C, 1], f32)
            nc.sync.dma_start(b64[:], fcb[:1, 0:1].broadcast_to((C, 1)))

            cnt_i = setup.tile([C, 1], i32)
            nc.vector.tensor_tensor(cnt_i[:], pos_sb[:, 1:2], pos_sb[:, 0:1], Alu.subtract)
            nc.vector.tensor_scalar_add(cnt_i[:], cnt_i[:], 1)
            cnt_f = setup.tile([C, 1], f32)
            nc.vector.tensor_copy(cnt_f[:], cnt_i[:])
            rcp = setup.tile([C, 1], f32)
            nc.vector.reciprocal(rcp[:], cnt_f[:])

            # ---- epilogue: s[c] = sum_h pooled*w; out = s/cnt + bias ----
            scratch = setup.tile([C, H], f32)
            s_sb = setup.tile([C, 1], f32)
            nc.vector.scalar_tensor_tensor(
                scratch[:], pooled[:], 1.0, w64[:], Alu.mult, Alu.mult,
                accum_out=s_sb[:],
            )
            res = setup.tile([C, 1], f32)
            nc.vector.scalar_tensor_tensor(
                res[:], s_sb[:], rcp[:], b64[:], Alu.mult, Alu.add
            )
            nc.sync.dma_start(outd[:], res[:])

    nc.compile()
    return nc


def kernel(feature, fc_weight, fc_bias, position_list):
    from concourse import bass_utils

    feature = np.asarray(feature, dtype=np.float32)
    fc_weight = np.asarray(fc_weight, dtype=np.float32)
    fc_bias = np.asarray(fc_bias, dtype=np.float32).reshape(1, 1)
    position_list = np.asarray(position_list, dtype=np.int32)

    nc = _CACHE.get("nc")
    if nc is None:
        nc = _build_nc()
        _CACHE["nc"] = nc

    in_maps = [
        {
            "feature": np.ascontiguousarray(feature[b]),
            "pos": np.ascontiguousarray(position_list[b]),
            "fc_w": fc_weight,
            "fc_b": fc_bias,
        }
        for b in range(B)
    ]
    res = bass_utils.run_bass_kernel_spmd(nc, in_maps, list(range(B)))
    out = np.concatenate([res.results[b]["out"] for b in range(B)], axis=0)
    return out.astype(np.float32)
